# revision 3
# baseline (speedup 1.0000x reference)
"""BinTokenizer kernel for Trainium2 (8 NeuronCores, data-parallel).

reference math: tokens = searchsorted(thresholds, clip(x, eps, 1-eps), 'right') - 1
with thresholds = linspace(0, 1, 257) in float32, which is exactly i/256.
For that uniform grid the search reduces to floor(x * 256): x*256 is exact in
f32 (power-of-two scale), and the f32->int32 convert on DVE truncates toward
zero, so a single tensor_scalar (mult 256, min 255) per element reproduces the
reference bit-exactly.  Each core handles 8 of the 64 batch rows.
"""

import sys

sys.path.insert(0, "/opt/trn_rl_repo")

import numpy as np

N_CORES = 8
B, T, D = 64, 4096, 512
PER_CORE = (B // N_CORES) * T * D  # 16,777,216 elements per core
P = 128                            # SBUF partitions
M = 8192                           # tile free dim (32 KiB/partition fp32)
ROWS = PER_CORE // M               # 2048 rows of M
NTILES = ROWS // P                 # 16 tiles of [128, 8192]

LAST_RESULT = None  # BassKernelResults of the most recent run (for test.py)

_program_cache = {}


def _build(scale: float, t0: float):
    import concourse.bass as bass
    import concourse.bacc as bacc
    import concourse.tile as tile
    from concourse import mybir

    # Bacc (not raw Bass): Tile emits multi-wait instructions, and only
    # Bacc's generate_event_semaphores pass splits them to the TRN2
    # one-wait-per-instruction limit walrus enforces.
    nc = bacc.Bacc("TRN2")
    x = nc.dram_tensor("x", [ROWS, M], mybir.dt.float32, kind="ExternalInput")
    y = nc.dram_tensor("y", [ROWS, M], mybir.dt.int32, kind="ExternalOutput")
    xt = x.rearrange("(n p) m -> n p m", p=P)
    yt = y.rearrange("(n p) m -> n p m", p=P)

    with tile.TileContext(nc) as tc:
        with tc.tile_pool(name="io_in", bufs=2) as in_pool, tc.tile_pool(
            name="io_out", bufs=2
        ) as out_pool:
            for i in range(NTILES):
                t_in = in_pool.tile([P, M], mybir.dt.float32, tag="in")
                nc.sync.dma_start(t_in[:], xt[i])
                t_out = out_pool.tile([P, M], mybir.dt.int32, tag="out")
                if t0 == 0.0:
                    # token = min(x*scale, nbins-1), truncated to int
                    nc.vector.tensor_scalar(
                        t_out[:],
                        t_in[:],
                        float(scale),
                        255.0,
                        mybir.AluOpType.mult,
                        mybir.AluOpType.min,
                    )
                else:
                    nc.vector.tensor_scalar(
                        t_out[:],
                        t_in[:],
                        float(t0),
                        float(scale),
                        mybir.AluOpType.subtract,
                        mybir.AluOpType.mult,
                    )
                # stores on the ACT HWDGE ring so they don't queue behind loads
                nc.scalar.dma_start(yt[i], t_out[:])

    nc.finalize()  # Bacc pass pipeline (reg alloc, event-sem wait splitting)
    return nc


def kernel(inputs: np.ndarray, thresholds: np.ndarray) -> np.ndarray:
    global LAST_RESULT
    from concourse.bass_utils import run_bass_kernel_spmd

    t = np.asarray(thresholds, dtype=np.float64)
    scale = float(1.0 / (t[1] - t[0]))
    t0 = float(t[0])

    key = (scale, t0)
    if key not in _program_cache:
        _program_cache[key] = _build(scale, t0)
    nc = _program_cache[key]

    x = np.asarray(inputs)
    if not x.flags.c_contiguous:
        x = np.ascontiguousarray(x)
    shards = x.reshape(N_CORES, ROWS, M)
    in_maps = [{"x": shards[c]} for c in range(N_CORES)]

    res = run_bass_kernel_spmd(nc, in_maps, list(range(N_CORES)))
    LAST_RESULT = res

    out = np.empty((N_CORES, ROWS, M), dtype=np.int32)
    for c in range(N_CORES):
        out[c] = res.results[c]["y"]
    return out.reshape(B, T, D)
